# revision 48
# baseline (speedup 1.0000x reference)
"""Multi-head attention (B=4, S=2048, D=768, H=12) on 8 Trainium2 cores.

Sharding: core c handles batch b=c//2 and heads [6*(c%2), 6*(c%2)+6).
Each core computes Q/K/V projections for its 6 heads (full sequence),
attention, and a partial out-projection (its 384 d_in columns of Wo).
Host gathers: out[b] = partial[2b] + partial[2b+1] + bo.

Device layout: feature-major QT/KT [d_out, token] (d_out on partitions,
2 heads per 128-partition group), token-major V [token, d_out]. Attention
computes scoresT [kpos, q] per head (row-packed pairs on the PE), exp on
ScalarE (PSUM->SBUF, scale=1/8 fused, no max subtraction needed: scores
are ~N(0,1)), PV col-packed (2 heads -> one [128, 512] psum), softmax
denominators via M=1 ones-matmuls, normalization by reciprocal +
partition-broadcast fused into the PV psum eviction.
"""

import os
import numpy as np
import ml_dtypes

import concourse.bass as bass
import concourse.tile as tile
from concourse import bacc, mybir
from concourse import bass_utils

B, S, D, H = 4, 2048, 768, 12
HD = D // H          # 64
SCALE = HD ** -0.5   # 0.125
NCORES = 8
HPC = H // 2         # heads per core = 6
G = HPC // 2         # head-pair groups per core = 3
QC = S // 512        # query chunks of 512 = 4
KT = S // 128        # key tiles of 128 = 16
TT = S // 128        # token tiles = 16
KO = D // 128        # d_in k-tiles = 6

F32 = mybir.dt.float32
BF16 = mybir.dt.bfloat16
DT = BF16
NPDT = ml_dtypes.bfloat16

_CACHE = {}
LAST_RESULTS = None


def _bcast_ap(ap: bass.AP, nparts: int) -> bass.AP:
    """Partition-broadcast view of a single-partition AP (step-0 partition dim)."""
    return bass.AP(tensor=ap.tensor, offset=ap.offset, ap=[[0, nparts], *ap.ap[1:]])


def _patch_act_tables():
    """Steer every Exp/Ln activation to the one table set containing both,
    so the kernel does a single ACT_TABLE_LOAD instead of thrashing between
    `exp_and_others` and `natural_log` (~1.3us per switch, 2/group)."""
    from concourse import hw_specs
    orig = hw_specs.get_activation_tables

    def patched(arch):
        t = dict(orig(arch))
        both = {mybir.ActivationFunctionType.Exp, mybir.ActivationFunctionType.Ln}
        for name in t:
            if name != "natural_log_exp_and_others":
                t[name] = set(t[name]) - both
        return t

    bacc.get_activation_tables = patched


def build_nc():
    _patch_act_tables()
    nc = bacc.Bacc(None, target_bir_lowering=False, debug=False)

    xT_d = nc.dram_tensor("xT", [128, KO, S], DT, kind="ExternalInput")
    wq_d = nc.dram_tensor("wqT", [128, KO, HPC * HD], DT, kind="ExternalInput")
    wk_d = nc.dram_tensor("wkT", [128, KO, HPC * HD], DT, kind="ExternalInput")
    wv_d = nc.dram_tensor("wvT", [128, KO, HPC * HD], DT, kind="ExternalInput")
    wo_d = nc.dram_tensor("woT", [128, G, D], DT, kind="ExternalInput")
    bq_d = nc.dram_tensor("bq", [128, G], F32, kind="ExternalInput")
    bk_d = nc.dram_tensor("bk", [128, G], F32, kind="ExternalInput")
    bv_d = nc.dram_tensor("bv", [128, HPC * HD], F32, kind="ExternalInput")
    out_d = nc.dram_tensor("out", [128, TT, D], F32, kind="ExternalOutput")

    with tile.TileContext(nc) as tc:
        with (
            tc.tile_pool(name="consts", bufs=1) as consts,
            tc.tile_pool(name="acts", bufs=1) as acts,
            tc.tile_pool(name="probs", bufs=2) as probs_pool,
            tc.tile_pool(name="small", bufs=2) as small,
            tc.tile_pool(name="ctxp", bufs=2) as ctxp,
            tc.tile_pool(name="ostage", bufs=3) as ostage_pool,
            tc.tile_pool(name="pp", bufs=2, space="PSUM") as pp,
            tc.tile_pool(name="scores", bufs=2, space="PSUM") as scores_pool,
            tc.tile_pool(name="ctxps", bufs=1, space="PSUM") as ctx_pool,
        ):
            # ---- load constants (small weights first so the first
            # projection matmuls aren't queued behind the 12MB xT load) ----
            wk = consts.tile([128, KO, HPC * HD], DT)
            nc.sync.dma_start(out=wk[:], in_=wk_d[:])
            bk = consts.tile([128, G], F32)
            nc.gpsimd.dma_start(out=bk[:], in_=bk_d[:])
            wq = consts.tile([128, KO, HPC * HD], DT)
            nc.gpsimd.dma_start(out=wq[:], in_=wq_d[:])
            bq = consts.tile([128, G], F32)
            nc.gpsimd.dma_start(out=bq[:], in_=bq_d[:])
            xT = consts.tile([128, KO, S], DT)
            for ko in range(KO):
                # alternate queues to double the effective load bandwidth
                eng = nc.sync if ko % 2 == 0 else nc.gpsimd
                eng.dma_start(out=xT[:, ko, :], in_=xT_d[:, ko, :])
            wv = consts.tile([128, KO, HPC * HD], DT)
            nc.gpsimd.dma_start(out=wv[:], in_=wv_d[:])
            bv = consts.tile([128, HPC * HD], F32)
            nc.gpsimd.dma_start(out=bv[:], in_=bv_d[:])
            wo = consts.tile([128, G, D], DT)
            nc.gpsimd.dma_start(out=wo[:], in_=wo_d[:])


            qt = acts.tile([128, G, S], DT)   # feature-major Q^T
            kt = acts.tile([128, G, S], DT)   # feature-major K^T
            # token-major V, 65 cols per head: col 64 = 1.0 so each PV
            # matmul's 65th output row accumulates the softmax denominator
            vt = acts.tile([128, TT, HPC, HD + 1], DT)
            nc.vector.memset(vt[:, :, :, HD:HD + 1], 1.0)

            def qk_proj(w, b, dst, g, qc):
                ps = pp.tile([128, 512], F32, tag="pp")
                for ko in range(KO):
                    nc.tensor.matmul(
                        ps[:],
                        lhsT=w[:, ko, g * 128:(g + 1) * 128],
                        rhs=xT[:, ko, qc * 512:(qc + 1) * 512],
                        start=(ko == 0),
                        stop=(ko == KO - 1),
                    )
                nc.vector.tensor_scalar_add(
                    out=dst[:, g, qc * 512:(qc + 1) * 512],
                    in0=ps[:],
                    scalar1=b[:, g:g + 1],
                )

            def v_proj(tt):
                ps = pp.tile([128, 512], F32, tag="pp")
                psv = ps[:, 0:HPC * HD]
                for ko in range(KO):
                    nc.tensor.matmul(
                        psv,
                        lhsT=xT[:, ko, tt * 128:(tt + 1) * 128],
                        rhs=wv[:, ko, :],
                        start=(ko == 0),
                        stop=(ko == KO - 1),
                    )
                nc.vector.tensor_add(
                    out=vt[:, tt, :, 0:HD],
                    in0=psv.rearrange("p (h d) -> p h d", h=HPC),
                    in1=bv[:].rearrange("p (h d) -> p h d", h=HPC),
                )

            # Up front only: K(g0) for all 4 q-chunks (the first QK sweep
            # needs the full K) -- ko-outer over a 4-bank psum so each
            # weight tile is loaded once for 4 matmuls -- plus Q(g0,qc0).
            # Everything else becomes PE filler inside the attention loop.
            kps0 = scores_pool.tile([128, 2, 512], F32, tag="st")
            kps1 = scores_pool.tile([128, 2, 512], F32, tag="st")
            kps = [kps0[:, 0, :], kps0[:, 1, :], kps1[:, 0, :], kps1[:, 1, :]]
            for ko in range(KO):
                for qc in range(QC):
                    nc.tensor.matmul(
                        kps[qc],
                        lhsT=wk[:, ko, 0:128],
                        rhs=xT[:, ko, qc * 512:(qc + 1) * 512],
                        start=(ko == 0),
                        stop=(ko == KO - 1),
                    )
            for qc in range(QC):
                nc.vector.tensor_scalar_add(
                    out=kt[:, 0, qc * 512:(qc + 1) * 512],
                    in0=kps[qc],
                    scalar1=bk[:, 0:1],
                )
            qk_proj(wq, bq, qt, 0, 0)

            # filler queues, scheduled into attention slots strictly before
            # their consumers: kq_g1 inside (qc0,g0), kq_g2 + Q(g0,rest)
            # inside (qc0,g1)
            kq_g1 = [("k", 1, qc) for qc in range(QC)] + [("q", 1, qc) for qc in range(QC)]
            kq_g2 = ([("k", 2, qc) for qc in range(QC)] + [("q", 2, qc) for qc in range(QC)]
                     + [("q", 0, qc) for qc in range(1, QC)])

            def run_filler(item):
                if item[0] == "v":
                    v_proj(item[1])
                elif item[0] == "k":
                    qk_proj(wk, bk, kt, item[1], item[2])
                else:
                    qk_proj(wq, bq, qt, item[1], item[2])

            # ---- attention + out-projection ----
            F32R = mybir.dt.float32r
            oproj_q = []  # deferred out-projection chunks (one per 2 tok tiles)

            def oproj(ctx_src, qc_src, tl):
                ost = ostage_pool.tile([128, D], F32)
                for nh in range(2):
                    po = pp.tile([128, 384], F32, tag="pp")
                    for g2_ in range(G):
                        nc.tensor.matmul(
                            po[:],
                            lhsT=ctx_src[:, g2_, tl * 128:(tl + 1) * 128],
                            rhs=wo[:, g2_, nh * 384:(nh + 1) * 384],
                            start=(g2_ == 0),
                            stop=(g2_ == G - 1),
                        )
                    nc.vector.tensor_copy(
                        out=ost[:, nh * 384:(nh + 1) * 384], in_=po[:])
                nc.gpsimd.dma_start(out=out_d[:, qc_src * 4 + tl, :], in_=ost[:])

            for qc in range(QC):
                ctx_t = ctxp.tile([128, G, 512], DT)
                for g in range(G):
                    # probs for both heads: [kpos-tile, head, q]
                    pr = probs_pool.tile([128, KT, 2, 512], DT, tag="pr")
                    cps = ctx_pool.tile([128, 2, 512], F32, tag="ctx")
                    qs = slice(qc * 512, (qc + 1) * 512)
                    def pv(t2):
                        st = (t2 == 0)
                        sp = (t2 == KT - 1)
                        nc.tensor.matmul(
                            cps[0:HD + 1, 0, :],
                            lhsT=vt[:, t2, 2 * g, :],
                            rhs=pr[:, t2, 0, :],
                            start=st, stop=sp,
                        )
                        nc.tensor.matmul(
                            cps[0:HD + 1, 1, :],
                            lhsT=vt[:, t2, 2 * g + 1, :],
                            rhs=pr[:, t2, 1, :],
                            start=st, stop=sp,
                        )

                    # PV trails QK/exp by PV_LAG tiles: the first PV waits on
                    # the previous group's psum eviction, and the PE queue is
                    # in-order -- the lag keeps QK work ahead of that stall.
                    # Scores: one 4-bank supertile, halves ping-ponged by the
                    # QK pairs, one [128,2048] exp per two kpos-tiles.
                    PV_LAG = 6
                    for t2 in range(KT):
                        # one supertile = both heads for kpos-tile t2; the
                        # row-packed pair (rows 0:64 / 64:128) is emitted
                        # adjacently so the PE can overlap the two streams
                        st_ = scores_pool.tile([128, 2, 512], F32, tag="st")
                        ks = slice(t2 * 128, (t2 + 1) * 128)
                        nc.tensor.matmul(
                            st_[:, 0, :],
                            lhsT=kt[0:64, g, ks],
                            rhs=qt[0:64, g, qs],
                            start=True, stop=True,
                        )
                        nc.tensor.matmul(
                            st_[:, 1, :],
                            lhsT=kt[64:128, g, ks],
                            rhs=qt[64:128, g, qs],
                            start=True, stop=True,
                        )
                        nc.scalar.activation(
                            out=pr[:, t2, :, :], in_=st_[:],
                            func=mybir.ActivationFunctionType.Exp, scale=SCALE,
                        )
                        # deferred projections / previous q-chunk's
                        # out-projection as PE filler under the exps
                        if qc == 0 and g == 0:
                            v_proj(t2)
                            if t2 % 2 == 0 and kq_g1:
                                run_filler(kq_g1.pop(0))
                        elif qc == 0 and g == 1 and kq_g2:
                            run_filler(kq_g2.pop(0))
                        elif oproj_q and t2 % 4 == 1:
                            oproj(*oproj_q.pop(0))
                        if t2 >= PV_LAG:
                            pv(t2 - PV_LAG)
                    for t2 in range(KT - PV_LAG, KT):
                        pv(t2)
                    # 1/denom: evict the two denominator rows (psum row 64),
                    # DMA-spread the 1024 values across 128 partitions so the
                    # DVE reciprocal runs full-lane (~0.2us instead of 8.5us),
                    # DMA back to partition 0, broadcast on idle GpSimd.
                    # Everything here is off the PE and ScalarE critical paths.
                    den = small.tile([128, 2, 512], F32, tag="den")
                    nc.vector.tensor_copy(out=den[64:65, :, :], in_=cps[64:65, :, :])
                    spread = small.tile([128, 8], F32, tag="spread")
                    nc.sync.dma_start(out=spread[:, :], in_=den[64:65, :, :])
                    rs = small.tile([128, 8], F32, tag="rspread")
                    nc.vector.reciprocal(out=rs[:], in_=spread[:])
                    rcp = small.tile([128, 2, 512], F32, tag="rcp")
                    nc.sync.dma_start(out=rcp[0:1, :, :], in_=rs[:, :])
                    bc = small.tile([64, 2, 512], F32, tag="bc")
                    nc.gpsimd.partition_broadcast(
                        out_ap=bc[0:64, :, :], in_ap=rcp[0:1, :, :], channels=64)
                    # normalize + evict: head A straight into ctx_t rows 0:64,
                    # head B via an SBUF stage + cross-partition DMA to 64:128
                    nc.vector.tensor_mul(
                        out=ctx_t[0:64, g, :], in0=cps[0:64, 0, :], in1=bc[0:64, 0, :])
                    stgB = small.tile([128, 512], DT, tag="stgB")
                    nc.vector.tensor_mul(
                        out=stgB[0:64, :], in0=cps[0:64, 1, :], in1=bc[0:64, 1, :])
                    nc.sync.dma_start(out=ctx_t[64:128, g, :], in_=stgB[0:64, :])

                # out-projection: defer into the next q-chunk's attention
                # slots as PE filler; the last q-chunk's runs at the end
                for tl in range(4):
                    if qc < QC - 1:
                        oproj_q.append((ctx_t, qc, tl))
                    else:
                        oproj(ctx_t, qc, tl)

    nc.compile()
    return nc


def _prep_inputs(x, Wq, bq, Wk, bk, Wv, bv, Wo):
    """Build the 8 per-core input maps (host-side shard + layout prep)."""
    def part_major(a):  # [(ko*128), m] -> [128, ko, m]
        k = a.shape[0] // 128
        return np.ascontiguousarray(
            a.reshape(k, 128, a.shape[1]).transpose(1, 0, 2))

    xT = [part_major(np.ascontiguousarray(x[b].T).astype(NPDT)) for b in range(B)]
    WqT, WkT, WvT = (np.ascontiguousarray(W.T.astype(NPDT)) for W in (Wq, Wk, Wv))
    WoT = np.ascontiguousarray(Wo.T.astype(NPDT))

    in_maps = []
    for c in range(NCORES):
        b = c // 2
        hs = (c % 2) * HPC * HD  # d slice start (384-wide)
        sl = slice(hs, hs + HPC * HD)
        in_maps.append({
            "xT": xT[b],
            "wqT": part_major(WqT[:, sl]),
            "wkT": part_major(WkT[:, sl]),
            "wvT": part_major(WvT[:, sl]),
            "woT": part_major(np.ascontiguousarray(WoT[sl, :])),
            "bq": np.ascontiguousarray(
                bq[sl].astype(np.float32).reshape(G, 128).T),
            "bk": np.ascontiguousarray(
                bk[sl].astype(np.float32).reshape(G, 128).T),
            "bv": np.ascontiguousarray(
                np.broadcast_to(bv[sl].astype(np.float32), (128, HPC * HD))),
        })
    return in_maps


def kernel(x, Wq, bq, Wk, bk, Wv, bv, Wo, bo):
    global LAST_RESULTS
    x, Wq, bq, Wk, bk, Wv, bv, Wo, bo = (
        np.asarray(a) for a in (x, Wq, bq, Wk, bk, Wv, bv, Wo, bo))
    if "nc" not in _CACHE:
        _CACHE["nc"] = build_nc()
    nc = _CACHE["nc"]
    in_maps = _prep_inputs(x, Wq, bq, Wk, bk, Wv, bv, Wo)
    res = bass_utils.run_bass_kernel_spmd(nc, in_maps, core_ids=list(range(NCORES)))
    LAST_RESULTS = res
    out = np.empty((B, S, D), np.float32)
    for b in range(B):
        p0 = res.results[2 * b]["out"].transpose(1, 0, 2).reshape(S, D)
        p1 = res.results[2 * b + 1]["out"].transpose(1, 0, 2).reshape(S, D)
        out[b] = p0 + p1 + bo.astype(np.float32)
    return out


if __name__ == "__main__":
    rng = np.random.default_rng(0)
    ins = {
        "x": rng.standard_normal((B, S, D), dtype=np.float32),
        "Wq": (rng.standard_normal((D, D), dtype=np.float32) * D ** -0.5),
        "Wk": (rng.standard_normal((D, D), dtype=np.float32) * D ** -0.5),
        "Wv": (rng.standard_normal((D, D), dtype=np.float32) * D ** -0.5),
        "Wo": (rng.standard_normal((D, D), dtype=np.float32) * D ** -0.5),
        "bq": rng.standard_normal(D, dtype=np.float32) * 0.01,
        "bk": rng.standard_normal(D, dtype=np.float32) * 0.01,
        "bv": rng.standard_normal(D, dtype=np.float32) * 0.01,
        "bo": rng.standard_normal(D, dtype=np.float32) * 0.01,
    }
    out = kernel(**ins)
    print("kernel ran, out:", out.shape, out.dtype, float(np.abs(out).mean()))


# revision 49
# speedup vs baseline: 1.0222x; 1.0222x over previous
"""Multi-head attention (B=4, S=2048, D=768, H=12) on 8 Trainium2 cores.

Sharding: core c handles batch b=c//2 and heads [6*(c%2), 6*(c%2)+6).
Each core computes Q/K/V projections for its 6 heads (full sequence),
attention, and a partial out-projection (its 384 d_in columns of Wo).
Host gathers: out[b] = partial[2b] + partial[2b+1] + bo.

Device layout: feature-major QT/KT [d_out, token] (d_out on partitions,
2 heads per 128-partition group), token-major V [token, d_out]. Attention
computes scoresT [kpos, q] per head (row-packed pairs on the PE), exp on
ScalarE (PSUM->SBUF, scale=1/8 fused, no max subtraction needed: scores
are ~N(0,1)), PV col-packed (2 heads -> one [128, 512] psum), softmax
denominators via M=1 ones-matmuls, normalization by reciprocal +
partition-broadcast fused into the PV psum eviction.
"""

import os
import numpy as np
import ml_dtypes

import concourse.bass as bass
import concourse.tile as tile
from concourse import bacc, mybir
from concourse import bass_utils

B, S, D, H = 4, 2048, 768, 12
HD = D // H          # 64
SCALE = HD ** -0.5   # 0.125
NCORES = 8
HPC = H // 2         # heads per core = 6
G = HPC // 2         # head-pair groups per core = 3
QC = S // 512        # query chunks of 512 = 4
KT = S // 128        # key tiles of 128 = 16
TT = S // 128        # token tiles = 16
KO = D // 128        # d_in k-tiles = 6

F32 = mybir.dt.float32
BF16 = mybir.dt.bfloat16
DT = BF16
NPDT = ml_dtypes.bfloat16

_CACHE = {}
LAST_RESULTS = None


def _bcast_ap(ap: bass.AP, nparts: int) -> bass.AP:
    """Partition-broadcast view of a single-partition AP (step-0 partition dim)."""
    return bass.AP(tensor=ap.tensor, offset=ap.offset, ap=[[0, nparts], *ap.ap[1:]])


def _patch_act_tables():
    """Steer every Exp/Ln activation to the one table set containing both,
    so the kernel does a single ACT_TABLE_LOAD instead of thrashing between
    `exp_and_others` and `natural_log` (~1.3us per switch, 2/group)."""
    from concourse import hw_specs
    orig = hw_specs.get_activation_tables

    def patched(arch):
        t = dict(orig(arch))
        both = {mybir.ActivationFunctionType.Exp, mybir.ActivationFunctionType.Ln}
        for name in t:
            if name != "natural_log_exp_and_others":
                t[name] = set(t[name]) - both
        return t

    bacc.get_activation_tables = patched


def build_nc():
    _patch_act_tables()
    nc = bacc.Bacc(None, target_bir_lowering=False, debug=False)

    xT_d = nc.dram_tensor("xT", [128, KO, S], DT, kind="ExternalInput")
    wq_d = nc.dram_tensor("wqT", [128, KO, HPC * HD], DT, kind="ExternalInput")
    wk_d = nc.dram_tensor("wkT", [128, KO, HPC * HD], DT, kind="ExternalInput")
    wv_d = nc.dram_tensor("wvT", [128, KO, HPC * HD], DT, kind="ExternalInput")
    wo_d = nc.dram_tensor("woT", [128, G, D], DT, kind="ExternalInput")
    bq_d = nc.dram_tensor("bq", [128, G], F32, kind="ExternalInput")
    bk_d = nc.dram_tensor("bk", [128, G], F32, kind="ExternalInput")
    bv_d = nc.dram_tensor("bv", [128, HPC * HD], F32, kind="ExternalInput")
    out_d = nc.dram_tensor("out", [128, TT, D], F32, kind="ExternalOutput")

    with tile.TileContext(nc) as tc:
        with (
            tc.tile_pool(name="consts", bufs=1) as consts,
            tc.tile_pool(name="acts", bufs=1) as acts,
            tc.tile_pool(name="probs", bufs=2) as probs_pool,
            tc.tile_pool(name="small", bufs=2) as small,
            tc.tile_pool(name="ctxp", bufs=2) as ctxp,
            tc.tile_pool(name="ostage", bufs=3) as ostage_pool,
            tc.tile_pool(name="pp", bufs=2, space="PSUM") as pp,
            tc.tile_pool(name="scores", bufs=2, space="PSUM") as scores_pool,
            tc.tile_pool(name="ctxps", bufs=1, space="PSUM") as ctx_pool,
        ):
            # ---- load constants (small weights first so the first
            # projection matmuls aren't queued behind the 12MB xT load) ----
            wk = consts.tile([128, KO, HPC * HD], DT)
            nc.sync.dma_start(out=wk[:], in_=wk_d[:])
            bk = consts.tile([128, G], F32)
            nc.gpsimd.dma_start(out=bk[:], in_=bk_d[:])
            wq = consts.tile([128, KO, HPC * HD], DT)
            nc.gpsimd.dma_start(out=wq[:], in_=wq_d[:])
            bq = consts.tile([128, G], F32)
            nc.gpsimd.dma_start(out=bq[:], in_=bq_d[:])
            xT = consts.tile([128, KO, S], DT)
            for ko in range(KO):
                # alternate queues to double the effective load bandwidth
                eng = nc.sync if ko % 2 == 0 else nc.gpsimd
                eng.dma_start(out=xT[:, ko, :], in_=xT_d[:, ko, :])
            wv = consts.tile([128, KO, HPC * HD], DT)
            nc.gpsimd.dma_start(out=wv[:], in_=wv_d[:])
            bv = consts.tile([128, HPC * HD], F32)
            nc.gpsimd.dma_start(out=bv[:], in_=bv_d[:])
            wo = consts.tile([128, G, D], DT)
            nc.gpsimd.dma_start(out=wo[:], in_=wo_d[:])


            qt = acts.tile([128, G, S], DT)   # feature-major Q^T
            kt = acts.tile([128, G, S], DT)   # feature-major K^T
            # token-major V, 65 cols per head: col 64 = 1.0 so each PV
            # matmul's 65th output row accumulates the softmax denominator
            vt = acts.tile([128, TT, HPC, HD + 1], DT)
            nc.vector.memset(vt[:, :, :, HD:HD + 1], 1.0)

            def qk_proj(w, b, dst, g, qc):
                ps = pp.tile([128, 512], F32, tag="pp")
                for ko in range(KO):
                    nc.tensor.matmul(
                        ps[:],
                        lhsT=w[:, ko, g * 128:(g + 1) * 128],
                        rhs=xT[:, ko, qc * 512:(qc + 1) * 512],
                        start=(ko == 0),
                        stop=(ko == KO - 1),
                    )
                nc.vector.tensor_scalar_add(
                    out=dst[:, g, qc * 512:(qc + 1) * 512],
                    in0=ps[:],
                    scalar1=b[:, g:g + 1],
                )

            def v_proj(tt):
                ps = pp.tile([128, 512], F32, tag="pp")
                psv = ps[:, 0:HPC * HD]
                for ko in range(KO):
                    nc.tensor.matmul(
                        psv,
                        lhsT=xT[:, ko, tt * 128:(tt + 1) * 128],
                        rhs=wv[:, ko, :],
                        start=(ko == 0),
                        stop=(ko == KO - 1),
                    )
                nc.vector.tensor_add(
                    out=vt[:, tt, :, 0:HD],
                    in0=psv.rearrange("p (h d) -> p h d", h=HPC),
                    in1=bv[:].rearrange("p (h d) -> p h d", h=HPC),
                )

            # Up front only: K(g0) for all 4 q-chunks (the first QK sweep
            # needs the full K) -- ko-outer over a 4-bank psum so each
            # weight tile is loaded once for 4 matmuls -- plus Q(g0,qc0).
            # Everything else becomes PE filler inside the attention loop.
            kps0 = scores_pool.tile([128, 2, 512], F32, tag="st")
            kps1 = scores_pool.tile([128, 2, 512], F32, tag="st")
            kps = [kps0[:, 0, :], kps0[:, 1, :], kps1[:, 0, :], kps1[:, 1, :]]
            for ko in range(KO):
                for qc in range(QC):
                    nc.tensor.matmul(
                        kps[qc],
                        lhsT=wk[:, ko, 0:128],
                        rhs=xT[:, ko, qc * 512:(qc + 1) * 512],
                        start=(ko == 0),
                        stop=(ko == KO - 1),
                    )
            for qc in range(QC):
                nc.vector.tensor_scalar_add(
                    out=kt[:, 0, qc * 512:(qc + 1) * 512],
                    in0=kps[qc],
                    scalar1=bk[:, 0:1],
                )
            qk_proj(wq, bq, qt, 0, 0)

            # filler queues, scheduled into attention slots strictly before
            # their consumers: kq_g1 inside (qc0,g0), kq_g2 + Q(g0,rest)
            # inside (qc0,g1)
            kq_g1 = [("k", 1, qc) for qc in range(QC)] + [("q", 1, qc) for qc in range(QC)]
            kq_g2 = ([("k", 2, qc) for qc in range(QC)] + [("q", 2, qc) for qc in range(QC)]
                     + [("q", 0, qc) for qc in range(1, QC)])

            def run_filler(item):
                if item[0] == "v":
                    v_proj(item[1])
                elif item[0] == "k":
                    qk_proj(wk, bk, kt, item[1], item[2])
                else:
                    qk_proj(wq, bq, qt, item[1], item[2])

            # ---- attention + out-projection ----
            F32R = mybir.dt.float32r
            oproj_q = []  # deferred out-projection chunks (one per 2 tok tiles)

            def oproj(ctx_src, qc_src, tl):
                ost = ostage_pool.tile([128, D], F32)
                for nh in range(2):
                    po = pp.tile([128, 384], F32, tag="pp")
                    for g2_ in range(G):
                        nc.tensor.matmul(
                            po[:],
                            lhsT=ctx_src[:, g2_, tl * 128:(tl + 1) * 128],
                            rhs=wo[:, g2_, nh * 384:(nh + 1) * 384],
                            start=(g2_ == 0),
                            stop=(g2_ == G - 1),
                        )
                    nc.vector.tensor_copy(
                        out=ost[:, nh * 384:(nh + 1) * 384], in_=po[:])
                nc.gpsimd.dma_start(out=out_d[:, qc_src * 4 + tl, :], in_=ost[:])

            for qc in range(QC):
                ctx_t = ctxp.tile([128, G, 512], DT)
                for g in range(G):
                    # probs for both heads: [kpos-tile, head, q]
                    pr = probs_pool.tile([128, KT, 2, 512], DT, tag="pr")
                    cps = ctx_pool.tile([128, 2, 512], F32, tag="ctx")
                    qs = slice(qc * 512, (qc + 1) * 512)
                    def pv(t2):
                        st = (t2 == 0)
                        sp = (t2 == KT - 1)
                        nc.tensor.matmul(
                            cps[0:HD + 1, 0, :],
                            lhsT=vt[:, t2, 2 * g, :],
                            rhs=pr[:, t2, 0, :],
                            start=st, stop=sp,
                        )
                        nc.tensor.matmul(
                            cps[0:HD + 1, 1, :],
                            lhsT=vt[:, t2, 2 * g + 1, :],
                            rhs=pr[:, t2, 1, :],
                            start=st, stop=sp,
                        )

                    # PV trails QK/exp by PV_LAG tiles: the first PV waits on
                    # the previous group's psum eviction, and the PE queue is
                    # in-order -- the lag keeps QK work ahead of that stall.
                    # Scores: one 4-bank supertile, halves ping-ponged by the
                    # QK pairs, one [128,2048] exp per two kpos-tiles.
                    PV_LAG = 4
                    for t2 in range(KT):
                        # one supertile = both heads for kpos-tile t2; the
                        # row-packed pair (rows 0:64 / 64:128) is emitted
                        # adjacently so the PE can overlap the two streams
                        st_ = scores_pool.tile([128, 2, 512], F32, tag="st")
                        ks = slice(t2 * 128, (t2 + 1) * 128)
                        nc.tensor.matmul(
                            st_[:, 0, :],
                            lhsT=kt[0:64, g, ks],
                            rhs=qt[0:64, g, qs],
                            start=True, stop=True,
                        )
                        nc.tensor.matmul(
                            st_[:, 1, :],
                            lhsT=kt[64:128, g, ks],
                            rhs=qt[64:128, g, qs],
                            start=True, stop=True,
                        )
                        nc.scalar.activation(
                            out=pr[:, t2, :, :], in_=st_[:],
                            func=mybir.ActivationFunctionType.Exp, scale=SCALE,
                        )
                        # deferred projections / previous q-chunk's
                        # out-projection as PE filler under the exps
                        if qc == 0 and g == 0:
                            v_proj(t2)
                            if t2 % 2 == 0 and kq_g1:
                                run_filler(kq_g1.pop(0))
                        elif qc == 0 and g == 1 and kq_g2:
                            run_filler(kq_g2.pop(0))
                        elif oproj_q and t2 % 4 == 1:
                            oproj(*oproj_q.pop(0))
                        if t2 >= PV_LAG:
                            pv(t2 - PV_LAG)
                    for t2 in range(KT - PV_LAG, KT):
                        pv(t2)
                    # 1/denom: evict the two denominator rows (psum row 64),
                    # DMA-spread the 1024 values across 128 partitions so the
                    # DVE reciprocal runs full-lane (~0.2us instead of 8.5us),
                    # DMA back to partition 0, broadcast on idle GpSimd.
                    # Everything here is off the PE and ScalarE critical paths.
                    den = small.tile([128, 2, 512], F32, tag="den")
                    nc.vector.tensor_copy(out=den[64:65, :, :], in_=cps[64:65, :, :])
                    spread = small.tile([128, 8], F32, tag="spread")
                    nc.sync.dma_start(out=spread[:, :], in_=den[64:65, :, :])
                    rs = small.tile([128, 8], F32, tag="rspread")
                    nc.vector.reciprocal(out=rs[:], in_=spread[:])
                    rcp = small.tile([128, 2, 512], F32, tag="rcp")
                    nc.sync.dma_start(out=rcp[0:1, :, :], in_=rs[:, :])
                    bc = small.tile([64, 2, 512], F32, tag="bc")
                    nc.gpsimd.partition_broadcast(
                        out_ap=bc[0:64, :, :], in_ap=rcp[0:1, :, :], channels=64)
                    # normalize + evict: head A straight into ctx_t rows 0:64,
                    # head B via an SBUF stage + cross-partition DMA to 64:128
                    nc.vector.tensor_mul(
                        out=ctx_t[0:64, g, :], in0=cps[0:64, 0, :], in1=bc[0:64, 0, :])
                    stgB = small.tile([128, 512], DT, tag="stgB")
                    nc.vector.tensor_mul(
                        out=stgB[0:64, :], in0=cps[0:64, 1, :], in1=bc[0:64, 1, :])
                    nc.sync.dma_start(out=ctx_t[64:128, g, :], in_=stgB[0:64, :])

                # out-projection: defer into the next q-chunk's attention
                # slots as PE filler; the last q-chunk's runs at the end
                for tl in range(4):
                    if qc < QC - 1:
                        oproj_q.append((ctx_t, qc, tl))
                    else:
                        oproj(ctx_t, qc, tl)

    nc.compile()
    return nc


def _prep_inputs(x, Wq, bq, Wk, bk, Wv, bv, Wo):
    """Build the 8 per-core input maps (host-side shard + layout prep)."""
    def part_major(a):  # [(ko*128), m] -> [128, ko, m]
        k = a.shape[0] // 128
        return np.ascontiguousarray(
            a.reshape(k, 128, a.shape[1]).transpose(1, 0, 2))

    xT = [part_major(np.ascontiguousarray(x[b].T).astype(NPDT)) for b in range(B)]
    WqT, WkT, WvT = (np.ascontiguousarray(W.T.astype(NPDT)) for W in (Wq, Wk, Wv))
    WoT = np.ascontiguousarray(Wo.T.astype(NPDT))

    in_maps = []
    for c in range(NCORES):
        b = c // 2
        hs = (c % 2) * HPC * HD  # d slice start (384-wide)
        sl = slice(hs, hs + HPC * HD)
        in_maps.append({
            "xT": xT[b],
            "wqT": part_major(WqT[:, sl]),
            "wkT": part_major(WkT[:, sl]),
            "wvT": part_major(WvT[:, sl]),
            "woT": part_major(np.ascontiguousarray(WoT[sl, :])),
            "bq": np.ascontiguousarray(
                bq[sl].astype(np.float32).reshape(G, 128).T),
            "bk": np.ascontiguousarray(
                bk[sl].astype(np.float32).reshape(G, 128).T),
            "bv": np.ascontiguousarray(
                np.broadcast_to(bv[sl].astype(np.float32), (128, HPC * HD))),
        })
    return in_maps


def kernel(x, Wq, bq, Wk, bk, Wv, bv, Wo, bo):
    global LAST_RESULTS
    x, Wq, bq, Wk, bk, Wv, bv, Wo, bo = (
        np.asarray(a) for a in (x, Wq, bq, Wk, bk, Wv, bv, Wo, bo))
    if "nc" not in _CACHE:
        _CACHE["nc"] = build_nc()
    nc = _CACHE["nc"]
    in_maps = _prep_inputs(x, Wq, bq, Wk, bk, Wv, bv, Wo)
    res = bass_utils.run_bass_kernel_spmd(nc, in_maps, core_ids=list(range(NCORES)))
    LAST_RESULTS = res
    out = np.empty((B, S, D), np.float32)
    for b in range(B):
        p0 = res.results[2 * b]["out"].transpose(1, 0, 2).reshape(S, D)
        p1 = res.results[2 * b + 1]["out"].transpose(1, 0, 2).reshape(S, D)
        out[b] = p0 + p1 + bo.astype(np.float32)
    return out


if __name__ == "__main__":
    rng = np.random.default_rng(0)
    ins = {
        "x": rng.standard_normal((B, S, D), dtype=np.float32),
        "Wq": (rng.standard_normal((D, D), dtype=np.float32) * D ** -0.5),
        "Wk": (rng.standard_normal((D, D), dtype=np.float32) * D ** -0.5),
        "Wv": (rng.standard_normal((D, D), dtype=np.float32) * D ** -0.5),
        "Wo": (rng.standard_normal((D, D), dtype=np.float32) * D ** -0.5),
        "bq": rng.standard_normal(D, dtype=np.float32) * 0.01,
        "bk": rng.standard_normal(D, dtype=np.float32) * 0.01,
        "bv": rng.standard_normal(D, dtype=np.float32) * 0.01,
        "bo": rng.standard_normal(D, dtype=np.float32) * 0.01,
    }
    out = kernel(**ins)
    print("kernel ran, out:", out.shape, out.dtype, float(np.abs(out).mean()))


# revision 52
# speedup vs baseline: 1.0225x; 1.0003x over previous
"""Multi-head attention (B=4, S=2048, D=768, H=12) on 8 Trainium2 cores.

Sharding: core c handles batch b=c//2 and heads [6*(c%2), 6*(c%2)+6).
Each core computes Q/K/V projections for its 6 heads (full sequence),
attention, and a partial out-projection (its 384 d_in columns of Wo).
Host gathers: out[b] = partial[2b] + partial[2b+1] + bo.

Device layout: feature-major QT/KT [d_out, token] (d_out on partitions,
2 heads per 128-partition group), token-major V [token, d_out]. Per
(head-pair, q-chunk): scoresT [kpos, q] via row-packed matmul pairs
(2 heads concurrent on the PE, dstart ~4ns), exp on ScalarE straight
from 2-bank PSUM supertiles into bf16 probsT (scale=1/8 fused; no max
subtraction needed: scores ~N(0,1), fp32 headroom is ample), PV with a
65th all-ones V column so the softmax denominator accumulates as psum
row 64 for free. 1/denom: DMA-spread the denominator row across 128
partitions, DVE reciprocal, DMA back, partition-broadcast on GpSimd,
multiply fused into the PV psum eviction (head B hops partitions via a
64KB SBUF-SBUF DMA). QKV/out projections and the out-projection of the
previous q-chunk are interleaved into the attention loop as PE filler
so the ScalarE-paced stretches keep the PE busy (HAM stays warm).
Weight loads are amortized ko-outer where psum banks allow.
"""

import os
import numpy as np
import ml_dtypes

import concourse.bass as bass
import concourse.tile as tile
from concourse import bacc, mybir
from concourse import bass_utils

B, S, D, H = 4, 2048, 768, 12
HD = D // H          # 64
SCALE = HD ** -0.5   # 0.125
NCORES = 8
HPC = H // 2         # heads per core = 6
G = HPC // 2         # head-pair groups per core = 3
QC = S // 512        # query chunks of 512 = 4
KT = S // 128        # key tiles of 128 = 16
TT = S // 128        # token tiles = 16
KO = D // 128        # d_in k-tiles = 6

F32 = mybir.dt.float32
BF16 = mybir.dt.bfloat16
DT = BF16
NPDT = ml_dtypes.bfloat16

_CACHE = {}
LAST_RESULTS = None


def _patch_act_tables():
    """Steer every Exp/Ln activation to the one table set containing both,
    so the kernel does a single ACT_TABLE_LOAD instead of thrashing between
    `exp_and_others` and `natural_log` (~1.3us per switch, 2/group)."""
    from concourse import hw_specs
    orig = hw_specs.get_activation_tables

    def patched(arch):
        t = dict(orig(arch))
        both = {mybir.ActivationFunctionType.Exp, mybir.ActivationFunctionType.Ln}
        for name in t:
            if name != "natural_log_exp_and_others":
                t[name] = set(t[name]) - both
        return t

    bacc.get_activation_tables = patched


def build_nc():
    _patch_act_tables()
    nc = bacc.Bacc(None, target_bir_lowering=False, debug=False)

    xT_d = nc.dram_tensor("xT", [128, KO, S], DT, kind="ExternalInput")
    wq_d = nc.dram_tensor("wqT", [128, KO, HPC * HD], DT, kind="ExternalInput")
    wk_d = nc.dram_tensor("wkT", [128, KO, HPC * HD], DT, kind="ExternalInput")
    wv_d = nc.dram_tensor("wvT", [128, KO, HPC * HD], DT, kind="ExternalInput")
    wo_d = nc.dram_tensor("woT", [128, G, D], DT, kind="ExternalInput")
    bq_d = nc.dram_tensor("bq", [128, G], F32, kind="ExternalInput")
    bk_d = nc.dram_tensor("bk", [128, G], F32, kind="ExternalInput")
    bv_d = nc.dram_tensor("bv", [128, HPC * HD], F32, kind="ExternalInput")
    out_d = nc.dram_tensor("out", [128, TT, D], F32, kind="ExternalOutput")

    with tile.TileContext(nc) as tc:
        with (
            tc.tile_pool(name="consts", bufs=1) as consts,
            tc.tile_pool(name="acts", bufs=1) as acts,
            tc.tile_pool(name="probs", bufs=2) as probs_pool,
            tc.tile_pool(name="small", bufs=2) as small,
            tc.tile_pool(name="ctxp", bufs=2) as ctxp,
            tc.tile_pool(name="ostage", bufs=3) as ostage_pool,
            tc.tile_pool(name="pp", bufs=2, space="PSUM") as pp,
            tc.tile_pool(name="scores", bufs=2, space="PSUM") as scores_pool,
            tc.tile_pool(name="ctxps", bufs=1, space="PSUM") as ctx_pool,
        ):
            # ---- load constants (small weights first so the first
            # projection matmuls aren't queued behind the 12MB xT load) ----
            wk = consts.tile([128, KO, HPC * HD], DT)
            nc.sync.dma_start(out=wk[:], in_=wk_d[:])
            bk = consts.tile([128, G], F32)
            nc.gpsimd.dma_start(out=bk[:], in_=bk_d[:])
            wq = consts.tile([128, KO, HPC * HD], DT)
            nc.gpsimd.dma_start(out=wq[:], in_=wq_d[:])
            bq = consts.tile([128, G], F32)
            nc.gpsimd.dma_start(out=bq[:], in_=bq_d[:])
            xT = consts.tile([128, KO, S], DT)
            for ko in range(KO):
                # alternate queues to double the effective load bandwidth
                eng = nc.sync if ko % 2 == 0 else nc.gpsimd
                eng.dma_start(out=xT[:, ko, :], in_=xT_d[:, ko, :])
            wv = consts.tile([128, KO, HPC * HD], DT)
            nc.gpsimd.dma_start(out=wv[:], in_=wv_d[:])
            bv = consts.tile([128, HPC * HD], F32)
            nc.gpsimd.dma_start(out=bv[:], in_=bv_d[:])
            wo = consts.tile([128, G, D], DT)
            nc.gpsimd.dma_start(out=wo[:], in_=wo_d[:])


            qt = acts.tile([128, G, S], DT)   # feature-major Q^T
            kt = acts.tile([128, G, S], DT)   # feature-major K^T
            # token-major V, 65 cols per head: col 64 = 1.0 so each PV
            # matmul's 65th output row accumulates the softmax denominator
            vt = acts.tile([128, TT, HPC, HD + 1], DT)
            nc.vector.memset(vt[:, :, :, HD:HD + 1], 1.0)

            def qk_proj(w, b, dst, g, qc):
                ps = pp.tile([128, 512], F32, tag="pp")
                for ko in range(KO):
                    nc.tensor.matmul(
                        ps[:],
                        lhsT=w[:, ko, g * 128:(g + 1) * 128],
                        rhs=xT[:, ko, qc * 512:(qc + 1) * 512],
                        start=(ko == 0),
                        stop=(ko == KO - 1),
                    )
                nc.vector.tensor_scalar_add(
                    out=dst[:, g, qc * 512:(qc + 1) * 512],
                    in0=ps[:],
                    scalar1=b[:, g:g + 1],
                )

            def v_proj(tt):
                ps = pp.tile([128, 512], F32, tag="pp")
                psv = ps[:, 0:HPC * HD]
                for ko in range(KO):
                    nc.tensor.matmul(
                        psv,
                        lhsT=xT[:, ko, tt * 128:(tt + 1) * 128],
                        rhs=wv[:, ko, :],
                        start=(ko == 0),
                        stop=(ko == KO - 1),
                    )
                nc.vector.tensor_add(
                    out=vt[:, tt, :, 0:HD],
                    in0=psv.rearrange("p (h d) -> p h d", h=HPC),
                    in1=bv[:].rearrange("p (h d) -> p h d", h=HPC),
                )

            # Up front only: K(g0) for all 4 q-chunks (the first QK sweep
            # needs the full K) -- ko-outer over a 4-bank psum so each
            # weight tile is loaded once for 4 matmuls -- plus Q(g0,qc0).
            # Everything else becomes PE filler inside the attention loop.
            kps0 = scores_pool.tile([128, 2, 512], F32, tag="st")
            kps1 = scores_pool.tile([128, 2, 512], F32, tag="st")
            kps = [kps0[:, 0, :], kps0[:, 1, :], kps1[:, 0, :], kps1[:, 1, :]]
            for ko in range(KO):
                for qc in range(QC):
                    nc.tensor.matmul(
                        kps[qc],
                        lhsT=wk[:, ko, 0:128],
                        rhs=xT[:, ko, qc * 512:(qc + 1) * 512],
                        start=(ko == 0),
                        stop=(ko == KO - 1),
                    )
            for qc in range(QC):
                nc.vector.tensor_scalar_add(
                    out=kt[:, 0, qc * 512:(qc + 1) * 512],
                    in0=kps[qc],
                    scalar1=bk[:, 0:1],
                )
            qk_proj(wq, bq, qt, 0, 0)

            # filler queues, scheduled into attention slots strictly before
            # their consumers: kq_g1 inside (qc0,g0), kq_g2 + Q(g0,rest)
            # inside (qc0,g1)
            kq_g1 = [("k", 1, qc) for qc in range(QC)] + [("q", 1, qc) for qc in range(QC)]
            kq_g2 = ([("k", 2, qc) for qc in range(QC)] + [("q", 2, qc) for qc in range(QC)]
                     + [("q", 0, qc) for qc in range(1, QC)])

            def run_filler(item):
                if item[0] == "v":
                    v_proj(item[1])
                elif item[0] == "k":
                    qk_proj(wk, bk, kt, item[1], item[2])
                else:
                    qk_proj(wq, bq, qt, item[1], item[2])

            # ---- attention + out-projection ----
            oproj_q = []  # deferred out-projection chunks (one per 2 tok tiles)

            def oproj(ctx_src, qc_src, tl):
                ost = ostage_pool.tile([128, D], F32)
                for nh in range(2):
                    po = pp.tile([128, 384], F32, tag="pp")
                    for g2_ in range(G):
                        nc.tensor.matmul(
                            po[:],
                            lhsT=ctx_src[:, g2_, tl * 128:(tl + 1) * 128],
                            rhs=wo[:, g2_, nh * 384:(nh + 1) * 384],
                            start=(g2_ == 0),
                            stop=(g2_ == G - 1),
                        )
                    nc.vector.tensor_copy(
                        out=ost[:, nh * 384:(nh + 1) * 384], in_=po[:])
                nc.gpsimd.dma_start(out=out_d[:, qc_src * 4 + tl, :], in_=ost[:])

            for qc in range(QC):
                ctx_t = ctxp.tile([128, G, 512], DT)
                for g in range(G):
                    # probs for both heads: [kpos-tile, head, q]
                    pr = probs_pool.tile([128, KT, 2, 512], DT, tag="pr")
                    cps = ctx_pool.tile([128, 2, 512], F32, tag="ctx")
                    qs = slice(qc * 512, (qc + 1) * 512)
                    def pv(t2):
                        st = (t2 == 0)
                        sp = (t2 == KT - 1)
                        nc.tensor.matmul(
                            cps[0:HD + 1, 0, :],
                            lhsT=vt[:, t2, 2 * g, :],
                            rhs=pr[:, t2, 0, :],
                            start=st, stop=sp,
                        )
                        nc.tensor.matmul(
                            cps[0:HD + 1, 1, :],
                            lhsT=vt[:, t2, 2 * g + 1, :],
                            rhs=pr[:, t2, 1, :],
                            start=st, stop=sp,
                        )

                    # PV trails QK/exp by PV_LAG tiles: the first PV waits on
                    # the previous group's psum eviction, and the PE queue is
                    # in-order -- the lag keeps QK work ahead of that stall.
                    # Scores: one 4-bank supertile, halves ping-ponged by the
                    # QK pairs, one [128,2048] exp per two kpos-tiles.
                    PV_LAG = 4
                    for t2 in range(KT):
                        # one supertile = both heads for kpos-tile t2; the
                        # row-packed pair (rows 0:64 / 64:128) is emitted
                        # adjacently so the PE can overlap the two streams
                        st_ = scores_pool.tile([128, 2, 512], F32, tag="st")
                        ks = slice(t2 * 128, (t2 + 1) * 128)
                        nc.tensor.matmul(
                            st_[:, 0, :],
                            lhsT=kt[0:64, g, ks],
                            rhs=qt[0:64, g, qs],
                            start=True, stop=True,
                        )
                        nc.tensor.matmul(
                            st_[:, 1, :],
                            lhsT=kt[64:128, g, ks],
                            rhs=qt[64:128, g, qs],
                            start=True, stop=True,
                        )
                        nc.scalar.activation(
                            out=pr[:, t2, :, :], in_=st_[:],
                            func=mybir.ActivationFunctionType.Exp, scale=SCALE,
                        )
                        # deferred projections / previous q-chunk's
                        # out-projection as PE filler under the exps
                        if qc == 0 and g == 0:
                            v_proj(t2)
                            if t2 % 2 == 0 and kq_g1:
                                run_filler(kq_g1.pop(0))
                        elif qc == 0 and g == 1 and kq_g2:
                            run_filler(kq_g2.pop(0))
                        elif oproj_q and t2 % 4 == 1:
                            oproj(*oproj_q.pop(0))
                        if t2 >= PV_LAG:
                            pv(t2 - PV_LAG)
                    for t2 in range(KT - PV_LAG, KT):
                        pv(t2)
                    # 1/denom: evict the two denominator rows (psum row 64),
                    # DMA-spread the 1024 values across 128 partitions so the
                    # DVE reciprocal runs full-lane (~0.2us instead of 8.5us),
                    # DMA back to partition 0, broadcast on idle GpSimd.
                    # Everything here is off the PE and ScalarE critical paths.
                    den = small.tile([128, 2, 512], F32, tag="den")
                    nc.vector.tensor_copy(out=den[64:65, :, :], in_=cps[64:65, :, :])
                    spread = small.tile([128, 8], F32, tag="spread")
                    nc.sync.dma_start(out=spread[:, :], in_=den[64:65, :, :])
                    rs = small.tile([128, 8], F32, tag="rspread")
                    nc.vector.reciprocal(out=rs[:], in_=spread[:])
                    rcp = small.tile([128, 2, 512], F32, tag="rcp")
                    nc.sync.dma_start(out=rcp[0:1, :, :], in_=rs[:, :])
                    bc = small.tile([64, 2, 512], F32, tag="bc")
                    nc.gpsimd.partition_broadcast(
                        out_ap=bc[0:64, :, :], in_ap=rcp[0:1, :, :], channels=64)
                    # normalize + evict: head A straight into ctx_t rows 0:64,
                    # head B via an SBUF stage + cross-partition DMA to 64:128
                    nc.vector.tensor_mul(
                        out=ctx_t[0:64, g, :], in0=cps[0:64, 0, :], in1=bc[0:64, 0, :])
                    stgB = small.tile([128, 512], DT, tag="stgB")
                    nc.vector.tensor_mul(
                        out=stgB[0:64, :], in0=cps[0:64, 1, :], in1=bc[0:64, 1, :])
                    nc.sync.dma_start(out=ctx_t[64:128, g, :], in_=stgB[0:64, :])

                # out-projection: defer into the next q-chunk's attention
                # slots as PE filler; the last q-chunk's runs at the end
                for tl in range(4):
                    if qc < QC - 1:
                        oproj_q.append((ctx_t, qc, tl))
                    else:
                        oproj(ctx_t, qc, tl)

    nc.compile()
    return nc


def _prep_inputs(x, Wq, bq, Wk, bk, Wv, bv, Wo):
    """Build the 8 per-core input maps (host-side shard + layout prep)."""
    def part_major(a):  # [(ko*128), m] -> [128, ko, m]
        k = a.shape[0] // 128
        return np.ascontiguousarray(
            a.reshape(k, 128, a.shape[1]).transpose(1, 0, 2))

    xT = [part_major(np.ascontiguousarray(x[b].T).astype(NPDT)) for b in range(B)]
    WqT, WkT, WvT = (np.ascontiguousarray(W.T.astype(NPDT)) for W in (Wq, Wk, Wv))
    WoT = np.ascontiguousarray(Wo.T.astype(NPDT))

    in_maps = []
    for c in range(NCORES):
        b = c // 2
        hs = (c % 2) * HPC * HD  # d slice start (384-wide)
        sl = slice(hs, hs + HPC * HD)
        in_maps.append({
            "xT": xT[b],
            "wqT": part_major(WqT[:, sl]),
            "wkT": part_major(WkT[:, sl]),
            "wvT": part_major(WvT[:, sl]),
            "woT": part_major(np.ascontiguousarray(WoT[sl, :])),
            "bq": np.ascontiguousarray(
                bq[sl].astype(np.float32).reshape(G, 128).T),
            "bk": np.ascontiguousarray(
                bk[sl].astype(np.float32).reshape(G, 128).T),
            "bv": np.ascontiguousarray(
                np.broadcast_to(bv[sl].astype(np.float32), (128, HPC * HD))),
        })
    return in_maps


def kernel(x, Wq, bq, Wk, bk, Wv, bv, Wo, bo):
    global LAST_RESULTS
    x, Wq, bq, Wk, bk, Wv, bv, Wo, bo = (
        np.asarray(a) for a in (x, Wq, bq, Wk, bk, Wv, bv, Wo, bo))
    if "nc" not in _CACHE:
        _CACHE["nc"] = build_nc()
    nc = _CACHE["nc"]
    in_maps = _prep_inputs(x, Wq, bq, Wk, bk, Wv, bv, Wo)
    res = bass_utils.run_bass_kernel_spmd(nc, in_maps, core_ids=list(range(NCORES)))
    LAST_RESULTS = res
    out = np.empty((B, S, D), np.float32)
    for b in range(B):
        p0 = res.results[2 * b]["out"].transpose(1, 0, 2).reshape(S, D)
        p1 = res.results[2 * b + 1]["out"].transpose(1, 0, 2).reshape(S, D)
        out[b] = p0 + p1 + bo.astype(np.float32)
    return out


if __name__ == "__main__":
    rng = np.random.default_rng(0)
    ins = {
        "x": rng.standard_normal((B, S, D), dtype=np.float32),
        "Wq": (rng.standard_normal((D, D), dtype=np.float32) * D ** -0.5),
        "Wk": (rng.standard_normal((D, D), dtype=np.float32) * D ** -0.5),
        "Wv": (rng.standard_normal((D, D), dtype=np.float32) * D ** -0.5),
        "Wo": (rng.standard_normal((D, D), dtype=np.float32) * D ** -0.5),
        "bq": rng.standard_normal(D, dtype=np.float32) * 0.01,
        "bk": rng.standard_normal(D, dtype=np.float32) * 0.01,
        "bv": rng.standard_normal(D, dtype=np.float32) * 0.01,
        "bo": rng.standard_normal(D, dtype=np.float32) * 0.01,
    }
    out = kernel(**ins)
    print("kernel ran, out:", out.shape, out.dtype, float(np.abs(out).mean()))
